# revision 10
# baseline (speedup 1.0000x reference)
"""Single-head causal self-attention (B=4, T=4096, C=1024, H=64) on 8 trn2 cores.

Sharding: core (b, f) handles batch b, fold f of the query rows.
Uniform SPMD program; fold differences are encoded in host-side data:
  - dataT [C, T] is the batch's data transposed; for fold B the 512-col
    half-groups are pairwise swapped so the program's fixed q-slices and
    mask slots line up for both folds.
  - masks [8, 128, 512]: m0..3 = ones (fold A) / zeros (fold B),
    m4..7 = causal staircases d=0,-128,-256,-384 (identical for both).

Per chunk j (width 512) the program attends to s in [0, E_j) with
E = [4096, 3072, 2048, 1024]; fold A's chunk-j queries are orig rows
[E_j-512, E_j), fold B's are [E_j-1024, E_j-512).

Device pipeline per core:
  kv^T = [Wk|Wv]^T @ dataT   (k^T rows 0:64, v^T rows 64:128)
  q^T  = Wq^T @ dataT[:, q-slices]
  v_aug[s] = PE-transpose(v^T block) + ones column  -> [128, 65]
  per chunk: S_T = k^T.T @ q^T (PSUM pairs [128,1024]) -> exp (ACT)
             -> mask-mult (DVE) -> out^T += v_aug^T @ W_T (row 64 = denom)
  epilogue:  transpose out^T, divide by denom, DMA out.
"""

import numpy as np
from contextlib import ExitStack

import concourse.bass as bass
from concourse import bacc
import concourse.mybir as mybir
import concourse.tile as tile
from concourse.bass_utils import run_bass_kernel_spmd
from concourse.masks import make_identity

B, T, C, H = 4, 4096, 1024, 64
CW = 512                      # q-chunk width
E = [4096, 3072, 2048, 1024]  # per-chunk program s-end
NS = [e // 128 for e in E]    # s-blocks per chunk
NCT = T // CW                 # 8 T-chunks for projections
NC_TILES = C // 128           # 8 C-tiles
T_TO_J = {7: 0, 5: 1, 3: 2, 1: 3}   # q-proj t-chunk -> attn chunk j
F32 = mybir.dt.float32
F32R = mybir.dt.float32r
BF16 = mybir.dt.bfloat16
SCALE = float(H) ** -0.5

_CACHE = {}


def _r(ap):
    return ap.bitcast(F32R)


def _build():
    nc = bacc.Bacc("TRN2", target_bir_lowering=False, debug=False)
    dataT = nc.dram_tensor("dataT", [C, T], BF16, kind="ExternalInput").ap()
    wkv = nc.dram_tensor("wkv", [C, 2 * H], BF16, kind="ExternalInput").ap()
    wq = nc.dram_tensor("wq", [C, H], BF16, kind="ExternalInput").ap()
    masks = nc.dram_tensor("masks", [8, 128, CW], BF16, kind="ExternalInput").ap()
    out = nc.dram_tensor("out", [2048, H], F32, kind="ExternalOutput").ap()

    with tile.TileContext(nc) as tc, ExitStack() as ctx:
        const = ctx.enter_context(tc.tile_pool(name="const", bufs=1))
        dt_pool = ctx.enter_context(tc.tile_pool(name="dt", bufs=16))
        persist = ctx.enter_context(tc.tile_pool(name="persist", bufs=1))
        w_pool = ctx.enter_context(tc.tile_pool(name="wt", bufs=3))
        ocsb_pool = ctx.enter_context(tc.tile_pool(name="ocsb", bufs=2))
        osb_pool = ctx.enter_context(tc.tile_pool(name="osb", bufs=3))
        rec_pool = ctx.enter_context(tc.tile_pool(name="rec", bufs=3))
        mm_ps = ctx.enter_context(tc.tile_pool(name="mmps", bufs=2, space="PSUM"))
        sp_ps = ctx.enter_context(tc.tile_pool(name="spps", bufs=2, space="PSUM"))
        sm_ps = ctx.enter_context(tc.tile_pool(name="smps", bufs=1, space="PSUM"))
        oa_ps = ctx.enter_context(tc.tile_pool(name="oaps", bufs=1, space="PSUM"))

        # ---- constants ----
        wkv_sb = const.tile([128, NC_TILES, 2 * H], BF16, tag="wkv")
        nc.gpsimd.dma_start(
            out=wkv_sb, in_=wkv.rearrange("(c p) m -> p c m", p=128))
        wq_sb = const.tile([128, NC_TILES, H], BF16, tag="wq")
        nc.gpsimd.dma_start(
            out=wq_sb, in_=wq.rearrange("(c p) m -> p c m", p=128))
        mask_sb = const.tile([128, 8, CW], BF16, tag="masks")
        nc.gpsimd.dma_start(
            out=mask_sb, in_=masks.rearrange("m p q -> p m q"))
        ident = const.tile([128, 128], BF16, tag="ident")
        identf = const.tile([128, 128], F32, tag="identf")
        make_identity(nc, identf)
        make_identity(nc, ident)

        # persistent per-core tensors
        kv_t = [persist.tile([128, CW], BF16, tag=f"kv{t}", name=f"kv{t}")
                for t in range(NCT)]
        q_sb = [persist.tile([64, CW], BF16, tag=f"q{j}", name=f"q{j}")
                for j in range(4)]
        v_aug = [persist.tile([128, H + 1], BF16, tag=f"va{s}", name=f"va{s}")
                 for s in range(T // 128)]

        def proj_chunk(t):
            dts = []
            for c in range(NC_TILES):
                dt_ = dt_pool.tile([128, CW], BF16, tag="dt")
                nc.gpsimd.dma_start(
                    out=dt_, in_=dataT[c * 128:(c + 1) * 128,
                                       t * CW:(t + 1) * CW])
                dts.append(dt_)
            kv_ps = mm_ps.tile([128, CW], F32, tag="mm")
            for c in range(NC_TILES):
                nc.tensor.matmul(kv_ps, wkv_sb[:, c, :], dts[c],
                                 start=(c == 0), stop=(c == NC_TILES - 1))
            nc.vector.tensor_copy(kv_t[t], kv_ps)
            # PE-transpose v^T blocks straight from kv_t partitions 64:128
            for sl in range(CW // 128):
                s = t * 4 + sl
                vt_ps = sm_ps.tile([128, H], BF16, tag="sm")
                nc.tensor.transpose(
                    vt_ps, kv_t[t][64:128, sl * 128:(sl + 1) * 128],
                    ident[64:128, 64:128])
                nc.vector.tensor_copy(v_aug[s][:, 0:H], vt_ps)
                nc.vector.memset(v_aug[s][:, H:H + 1], 1.0)
            if t in T_TO_J:
                j = T_TO_J[t]
                q_ps = mm_ps.tile([64, CW], F32, tag="mm")
                for c in range(NC_TILES):
                    nc.tensor.matmul(q_ps, wq_sb[:, c, :], dts[c],
                                     start=(c == 0), stop=(c == NC_TILES - 1))
                nc.vector.tensor_copy(q_sb[j], q_ps)

        def attn_chunk(j):
            npair = NS[j] // 2
            oacc = oa_ps.tile([H + 1, CW], F32, tag="oa")
            for p in range(npair):
                s0, s1 = 2 * p, 2 * p + 1
                s_ps = sp_ps.tile([128, 2 * CW], F32, tag="sp")
                for i, s in enumerate((s0, s1)):
                    nc.tensor.matmul(
                        s_ps[:, i * CW:(i + 1) * CW],
                        kv_t[s // 4][0:64, (s % 4) * 128:(s % 4 + 1) * 128],
                        q_sb[j], start=True, stop=True)
                w_sb = w_pool.tile([128, 2 * CW], BF16, tag="w")
                nc.scalar.activation(w_sb, s_ps,
                                     mybir.ActivationFunctionType.Exp,
                                     scale=SCALE)
                m = p - (npair - 4)
                if m >= 0:
                    nc.vector.tensor_mul(
                        w_sb, w_sb, mask_sb[:, 2 * m:2 * m + 2, :])
                for i, s in enumerate((s0, s1)):
                    nc.tensor.matmul(
                        oacc, v_aug[s], w_sb[:, i * CW:(i + 1) * CW],
                        start=(p == 0 and i == 0),
                        stop=(p == npair - 1 and i == 1))
            oc_sb = ocsb_pool.tile([H + 1, CW], F32, tag="oc")
            nc.vector.tensor_copy(oc_sb, oacc)
            for u in range(CW // 128):
                tr_ps = sm_ps.tile([128, H + 1], F32, tag="sm")
                nc.tensor.transpose(
                    tr_ps, oc_sb[:, u * 128:(u + 1) * 128],
                    identf[0:H + 1, 0:H + 1])
                rec = rec_pool.tile([128, 1], F32, tag="rec")
                nc.vector.reciprocal(rec, tr_ps[:, H:H + 1])
                osb = osb_pool.tile([128, H], F32, tag="ob")
                nc.vector.tensor_scalar_mul(osb, tr_ps[:, 0:H], rec)
                nc.gpsimd.dma_start(
                    out=out[j * CW + u * 128:j * CW + (u + 1) * 128, :],
                    in_=osb)

        # interleaved emission: attention chunk j becomes ready as soon as
        # its kv prefix + q slice are projected (j=3 needs t0,t1 only).
        proj_chunk(0)
        proj_chunk(1)
        attn_chunk(3)
        proj_chunk(2)
        proj_chunk(3)
        attn_chunk(2)
        proj_chunk(4)
        proj_chunk(5)
        attn_chunk(1)
        proj_chunk(6)
        proj_chunk(7)
        attn_chunk(0)

    nc.compile()
    return nc


def _host_inputs(data, Wk, Wq, Wv):
    """Build the 8 per-core input maps. Core id = b*2 + f."""
    import ml_dtypes
    bf16 = ml_dtypes.bfloat16
    wkv = np.ascontiguousarray(np.concatenate([Wk, Wv], axis=1)).astype(bf16)
    wq = np.ascontiguousarray(Wq).astype(bf16)

    stair = np.zeros((8, 128, CW), np.float32)
    ql = np.arange(CW)[None, :]
    sl = np.arange(128)[:, None]
    for i in range(4):
        stair[4 + i] = (ql >= sl + 128 * i).astype(np.float32)
    masks_A = stair.copy()
    masks_A[0:4] = 1.0
    masks_B = stair.copy()
    masks_B[0:4] = 0.0
    masks_A = masks_A.astype(bf16)
    masks_B = masks_B.astype(bf16)

    in_maps = []
    for b in range(B):
        dT = np.ascontiguousarray(data[b].T).astype(bf16)   # [C, T]
        # fold B: swap 512-col halves of each 1024-col group
        dT_B = np.ascontiguousarray(
            dT.reshape(C, 8, CW)[:, [1, 0, 3, 2, 5, 4, 7, 6], :].reshape(C, T))
        for f, (dmat, msk) in enumerate(((dT, masks_A), (dT_B, masks_B))):
            in_maps.append({"dataT": dmat, "wkv": wkv, "wq": wq,
                            "masks": np.ascontiguousarray(msk)})
    return in_maps


def _gather(results):
    out = np.empty((B, T, H), np.float32)
    for b in range(B):
        for f in range(2):
            oc = results[b * 2 + f]["out"]
            for j in range(4):
                q0 = E[j] - 512 - 512 * f
                out[b, q0:q0 + 512] = oc[j * CW:(j + 1) * CW]
    return out


def kernel(data, Wk, Wq, Wv, _trace=False, _tmpdir=None):
    data = np.asarray(data, np.float32)
    if "nc" not in _CACHE:
        _CACHE["nc"] = _build()
    nc = _CACHE["nc"]
    in_maps = _host_inputs(data, np.asarray(Wk, np.float32),
                           np.asarray(Wq, np.float32),
                           np.asarray(Wv, np.float32))
    res = run_bass_kernel_spmd(nc, in_maps, list(range(8)), trace=_trace,
                               tmpdir=_tmpdir)
    _CACHE["last"] = res
    return _gather(res.results)
